# revision 70
# baseline (speedup 1.0000x reference)
"""AdditiveAttention Trainium2 kernel (8 NeuronCores, SPMD, no collectives).

reference:
    q = queries @ Wq               (B,Q,H)
    k = keys @ Wk                  (B,K,H)
    scores[b,q,k] = sum_h wv[h] * tanh(q[b,q,h] + k[b,k,h])
    masked = where(arange(K) < valid_lens[b], scores, 0.0)
    attn = softmax(masked, -1)      # masked cols contribute e^0 = 1
    out = attn @ values             (B,Q,D)

Sharding: core c = (b, q_half) -> computes out[b, qh*128:(qh+1)*128, :].
Each core owns 128 queries x full K of one batch. Purely data-parallel,
no collectives, no cross-core reduction (strictly better here than the
flash-style K-split: each output element is computed exactly once).

Per-core structure (h=H=128 on partitions for the score stage;
ScalarE's tanh throughput, 1 elem/lane/cycle @ 1.2 GHz, is the roofline):
  - kpT[h, k], qpT[h, q] via PE matmuls (bf16 in, f32 accum); all inputs
    arrive host-packed as exact SBUF images so DMA descriptors are maximal;
    k-chunk-0 of kT split across both HWDGE rings, v/mask DMAs deferred
    behind the last kT piece (they'd steal SDMA packets at startup)
  - head 4 query rows: bias-fused tanh straight from the kp PSUM (ScalarE
    per-partition bias, no VectorE dependency, first two split per k-chunk)
    so the stream starts right after the first kp projection chunk
  - remaining rows in groups of 8: VectorE broadcast-adds kpT + qpT[:, q]
    (tensor_scalar, per-partition scalar, f32 2x mode, ~87us - hidden),
    then ONE ScalarE tanh over the fused group ([128, 8*KE]) amortizing
    the ~228-cycle ACT instruction overhead; groups de-ramp ..4,2,2 at the
    very end so the last rows' matmuls trail a short tanh
  - per q: 2 PE matmuls with a 32-wide "sliding window" stationary operand
    (wv at column q%32, zeros elsewhere) accumulate that q's scores into
    row q%32 (psum col-group base 32*g) of the part's PSUM tile
    -> dense scores[q, k]; wv is never reloaded per row
  - two row parts (96/32) in SEPARATE PSUM banks (same-bank PE-write +
    engine-read is a hardware race): part-0's entire epilogue - mask
    multiply (masked logits -> 0), exp with accum_out giving the softmax
    denominator for free, PE transposes E -> E_T in two banks, attn@V
    matmuls, 1/Z normalize, output DMA - runs hidden under part-1's tanh
    stream; only the last 32 rows' epilogue trails the loop (with dummy
    PE matmuls keeping the HAM clock warm through its mask/exp window)
  - k >= KE tail of attn@V uses an all-ones stationary operand (exp(0)=1)

KE = ceil(max(valid_lens)/128)*128 <= K: columns >= KE are masked in every
batch, so tanh/exp work shrinks to KE columns (kernel specializes the
compiled graph to the runtime valid_lens, cached per KE).

Measured (8 cores, neuron-profile exec_time, chip at full 1.2 GHz clock):
~128-132us at KE=896, ~144-147us at KE=1024 (ScalarE-busy floor ~114us at
KE=1024, ~100us at KE=896; ACT stream has zero mid-loop gaps). The chip
intermittently downclocks whole runs to 1.0/0.9 GHz (+20-30%). First
working version was 170.6us. rel err ~3e-3 (bf16 tanh/matmul operands,
f32 accumulation everywhere).
"""

import sys

sys.path.insert(0, "/opt/trn_rl_repo")

from contextlib import ExitStack

import numpy as np
import ml_dtypes

import concourse.bass as bass
import concourse.mybir as mybir
import concourse.tile as tile
from concourse import bacc
from concourse.bass_utils import run_bass_kernel_spmd
from concourse.masks import make_identity
from concourse.tile_rust import add_dep_helper

B, Q, K, D, H = 4, 256, 1024, 512, 128
QS = Q // 2  # queries per core
N_CORES = 8
F32 = mybir.dt.float32
BF16 = mybir.dt.bfloat16
BF16_NP = np.dtype(ml_dtypes.bfloat16)
WU_MM = 6  # PE warmup matmuls under the DMA shadow (more would delay the
# kp projections queued behind them in PE's FIFO; kp/qp continue the busy
# window so HAM still warms)


def build_graph(KE: int) -> bass.Bass:
    assert KE % 128 == 0 and 128 <= KE <= K
    DC = D // 128  # contraction chunks for the projections
    # n-chunks (<=512) of the score/exp free axis
    k_chunks = [(s, min(512, KE - s)) for s in range(0, KE, 512)]
    KC128 = KE // 128
    VC = K // 128
    HQ = QS // 2  # epilogue half

    H0 = 96  # rows finished early (hidden under the tanh stream)
    H1 = QS - H0

    nc = bacc.Bacc("TRN2", target_bir_lowering=False, debug=False)

    # all inputs arrive host-packed as the exact SBUF image ([128, N],
    # contiguous per partition) so every DMA runs at max descriptor size.
    # kT is additionally packed k-chunk-major so each k-chunk half is a
    # contiguous column range (split across the two HWDGE rings).
    kT_d = nc.declare_dram_parameter("kT", [128, DC * KE], BF16, isOutput=False)
    v_d = nc.declare_dram_parameter("v", [128, VC * D], BF16, isOutput=False)
    # small critical inputs concatenated per HWDGE ring (one DMA receipt each):
    # sy_small = wk || wvwin (sliding windows: col 30 / col 64+31 = wv)
    # sc_small = qT || wq
    sy_d = nc.declare_dram_parameter("sy_small", [128, DC * H + 128], BF16, isOutput=False)
    sc_d = nc.declare_dram_parameter("sc_small", [128, DC * QS + DC * H], BF16, isOutput=False)
    mask_d = nc.declare_dram_parameter("mask", [H0, KE], BF16, isOutput=False)
    out_d = nc.declare_dram_parameter("out", [QS, D], F32, isOutput=True)

    with tile.TileContext(nc) as tc, ExitStack() as ctx:
        const = ctx.enter_context(tc.tile_pool(name="const", bufs=1))
        work = ctx.enter_context(tc.tile_pool(name="work", bufs=1))
        tq_pool = ctx.enter_context(tc.tile_pool(name="tq", bufs=3))
        xa_pool = ctx.enter_context(tc.tile_pool(name="xa", bufs=3))
        pp = ctx.enter_context(tc.tile_pool(name="pp", bufs=1, space="PSUM"))
        scp = ctx.enter_context(tc.tile_pool(name="scp", bufs=1, space="PSUM"))
        tpp = ctx.enter_context(tc.tile_pool(name="tpp", bufs=2, space="PSUM"))
        pop = ctx.enter_context(tc.tile_pool(name="pop", bufs=1, space="PSUM"))

        # ---- load inputs (few big DMAs, spread over both HWDGE rings) ----
        kT_sb = const.tile([128, DC * KE], BF16, tag="kT")
        v_sb = const.tile([128, VC * D], BF16, tag="v")
        sy_sb = const.tile([128, DC * H + 128], BF16, tag="sy_small")
        sc_sb = const.tile([128, DC * QS + DC * H], BF16, tag="sc_small")
        mask_sb = const.tile([H0, KE], BF16, tag="mask")
        wk_sb = sy_sb[:, : DC * H]
        wvwin_sb = sy_sb[:, DC * H :]
        qT_sb = sc_sb[:, : DC * QS]
        wq_sb = sc_sb[:, DC * QS :]
        # smalls first (wk gates every kp matmul, qT/wq the bias path);
        # k-chunk-0 of kT is split across BOTH HWDGE rings so the first kp
        # chunk - which gates the first bias-fused tanh - lands in half the
        # time; chunk-major host packing keeps every piece contiguous
        kcut = DC * k_chunks[0][1]
        kq = kcut // 2
        nc.sync.dma_start(sy_sb[:], sy_d[:, :])
        nc.scalar.dma_start(sc_sb[:], sc_d[:, :])
        nc.sync.dma_start(kT_sb[:, :kq], kT_d[:, :kq])
        last_kt = nc.scalar.dma_start(kT_sb[:, kq:kcut], kT_d[:, kq:kcut])
        if kcut < DC * KE:
            kq2 = (kcut + DC * KE) // 2
            nc.sync.dma_start(kT_sb[:, kcut:kq2], kT_d[:, kcut:kq2])
            last_kt = nc.scalar.dma_start(kT_sb[:, kq2:], kT_d[:, kq2:])

        def kT_ci(ci, i):
            """d-chunk i of k-chunk ci, as packed: [base_ci + i*w, +w)."""
            base = DC * sum(w for _, w in k_chunks[:ci])
            w = k_chunks[ci][1]
            return kT_sb[:, base + i * w : base + (i + 1) * w]

        def v_c(i):
            return v_sb[:, i * D : (i + 1) * D]

        # ---- PE warmup burst (HAM un-throttle) under the DMA shadow ----
        wu_in = const.tile([128, 512], BF16, tag="wu_in")
        nc.gpsimd.memset(wu_in[:], 0.0)
        wu_ps = pop.tile([128, 512], F32, tag="po", name="wu_ps")
        for i in range(WU_MM):
            nc.tensor.matmul(
                wu_ps[:], wu_in[:, :128], wu_in[:], start=True, stop=True
            )

        # ---- projections: kpT[h, k] first (gates the tanh stream) ----
        kp_ps = pp.tile([H, KE], F32, tag="kp_ps")
        kp_sb = work.tile([H, KE], F32, tag="kp_sb")
        kp_c0_stop = None
        for ci, (s, w) in enumerate(k_chunks):
            for i in range(DC):
                mm = nc.tensor.matmul(
                    kp_ps[:, s : s + w],
                    wk_sb[:, i * H : (i + 1) * H],
                    kT_ci(ci, i),
                    start=(i == 0),
                    stop=(i == DC - 1),
                )
                if ci == 0 and i == DC - 1:
                    kp_c0_stop = mm
            nc.vector.tensor_copy(kp_sb[:, s : s + w], kp_ps[:, s : s + w])

        # v/mask are not needed until the epilogue (~110us in): defer their
        # DMA triggers behind the last kT piece so their 1.2MB doesn't steal
        # SDMA packets from the critical kT/qT/wq loads during startup
        # (dep on the DMA itself also forces trigger order in the sequencer)
        vd = nc.scalar.dma_start(v_sb[:], v_d[:, :])
        add_dep_helper(vd.ins, last_kt.ins, reason="defer v dma")
        md = nc.scalar.dma_start(mask_sb[:], mask_d[:, :])
        add_dep_helper(md.ins, last_kt.ins, reason="defer mask dma")
        qp_ps = scp.tile([H, QS], F32, tag="sc_ps", name="qp_ps")
        for i in range(DC):
            nc.tensor.matmul(
                qp_ps[:],
                wq_sb[:, i * H : (i + 1) * H],
                qT_sb[:, i * QS : (i + 1) * QS],
                start=(i == 0),
                stop=(i == DC - 1),
            )
        qp_sb = work.tile([H, QS], F32, tag="qp_sb")
        nc.vector.tensor_copy(qp_sb[:], qp_ps[:])

        ident = const.tile([128, 128], BF16, tag="ident")
        make_identity(nc, ident[:])
        ones_sb = const.tile([128, 128], BF16, tag="ones")
        nc.gpsimd.memset(ones_sb[:], 1.0)

        et_sb = work.tile([128, KC128 * 128], BF16, tag="et_sb")
        out_sb = work.tile([QS, D], F32, tag="out_sb")

        def scores_mm(q, tq_ap, sc_h):
            """score scatter matmuls for one query row from its tanh slice."""
            g, r = divmod(q if q < H0 else q - H0, 32)
            off = (30 - r) if r % 2 == 0 else (64 + 31 - r)
            win = wvwin_sb[:, off : off + 32]
            for s, w in k_chunks:
                nc.tensor.matmul(
                    sc_h[g * 32 : (g + 1) * 32, s : s + w],
                    win,
                    tq_ap[:, s : s + w],
                    start=(r == 0),
                    stop=(r == 31),
                    tile_position=(0, g * 32),
                )

        QG = 8  # max queries fused per ScalarE tanh instruction

        def group_sizes(n, deramp=False):
            """small first group then 8s; de-ramp ..4,2,1,1 at the tail so the
            last rows' score matmuls trail a single-row tanh"""
            tail = [4, 2, 1, 1] if (deramp and n >= 2 * QG) else []
            rem = n - sum(tail)
            sizes = [rem % QG] if rem % QG else []
            sizes += [QG] * (rem // QG)
            return sizes + tail

        def bias_tanh_single(q, sc_h, chunked):
            """Head query rows: bias-fused tanh straight from the kp PSUM -
            no DVE-add dependency, so the stream starts right after the kp
            matmuls (chunked: right after the FIRST kp chunk, overlapping the
            second chunk's DMA + matmuls in the other PSUM bank)."""
            tq = tq_pool.tile([H, QG * KE], BF16, tag="tq", name="tq")
            for s, w in k_chunks if chunked else [(0, KE)]:
                nc.scalar.activation(
                    tq[:, s : s + w],
                    kp_ps[:, s : s + w],
                    mybir.ActivationFunctionType.Tanh,
                    bias=qp_sb[:, q : q + 1],
                )
            scores_mm(q, tq[:, :KE], sc_h)

        def q_group_block(q0, ng, sc_of):
            """Broadcast-add on DVE (per-partition scalar), pure tanh on
            ScalarE over a fused group of query rows (amortizes the ~228-cycle
            ACT per-instruction overhead), then the score matmuls."""
            xa = xa_pool.tile([H, QG * KE], F32, tag="xa", name="xa")
            for j in range(ng):
                nc.vector.tensor_scalar_add(
                    xa[:, j * KE : (j + 1) * KE],
                    kp_sb[:],
                    qp_sb[:, q0 + j : q0 + j + 1],
                )
            tq = tq_pool.tile([H, QG * KE], BF16, tag="tq", name="tq")
            nc.scalar.activation(
                tq[:, : ng * KE], xa[:, : ng * KE], mybir.ActivationFunctionType.Tanh
            )
            for j in range(ng):
                scores_mm(q0 + j, tq[:, j * KE : (j + 1) * KE], sc_of(q0 + j))

        def epilogue_part(h, sc_h, po_h, r0, nr):
            """mask + exp + transpose + attn@V + normalize + store for query
            rows [r0, r0+nr).

            All tiles here live on partitions 0:nr (engines cannot shift
            partitions); the q-offset reappears as a column offset in et_sb
            and as the DRAM row offset of the output DMA.
            """
            msk_h = work.tile([nr, KE], F32, tag=f"msk{h}", name=f"msk{h}")
            e_h = work.tile([nr, KE], BF16, tag=f"e{h}", name=f"e{h}")
            z_h = work.tile([nr, 2], F32, tag=f"z{h}", name=f"z{h}")
            # mask+exp in k-halves so exp(half0) overlaps mask(half1)
            ecut = k_chunks[0][1]
            for ei, (es, ew) in enumerate([(0, ecut), (ecut, KE - ecut)]):
                if ew <= 0:
                    continue
                nc.vector.tensor_mul(
                    msk_h[:, es : es + ew],
                    sc_h[:, es : es + ew],
                    mask_sb[:nr, es : es + ew],
                )
                nc.scalar.activation(
                    e_h[:, es : es + ew],
                    msk_h[:, es : es + ew],
                    mybir.ActivationFunctionType.Exp,
                    accum_out=z_h[:, ei : ei + 1],
                )
            # transposes in two waves over two PSUM banks; each wave's evac
            # (DVE) overlaps the other wave's transposes (PE)
            W0 = KC128 // 2
            waves = [(0, W0), (W0, KC128 - W0)] if W0 else [(0, KC128)]
            for c0, ncw in waves:
                tp = tpp.tile([128, max(KC128 - W0, 1) * nr], BF16, tag="tp", name="tp")
                for j in range(ncw):
                    nc.tensor.transpose(
                        tp[:, j * nr : (j + 1) * nr],
                        e_h[:, (c0 + j) * 128 : (c0 + j + 1) * 128],
                        ident[:nr, :nr],
                    )
                nc.vector.tensor_copy(
                    et_sb[:]
                    .rearrange("p (c n) -> p c n", c=KC128)[
                        :, c0 : c0 + ncw, r0 : r0 + nr
                    ],
                    tp[:, : ncw * nr].rearrange("p (c n) -> p c n", c=ncw),
                )
            # attn @ V (tail k-chunks beyond KE use ones: exp(0) = 1)
            for c in range(VC):
                lhsT = (
                    et_sb[:, c * 128 + r0 : c * 128 + r0 + nr]
                    if c < KC128
                    else ones_sb[:, :nr]
                )
                nc.tensor.matmul(
                    po_h[:],
                    lhsT,
                    v_c(c),
                    start=(c == 0),
                    stop=(c == VC - 1),
                )
            # normalize + store (partition-aligned: po_h/z_h live on 0:nr)
            z2 = work.tile([nr, 1], F32, tag=f"z2_{h}", name=f"z2_{h}")
            nc.vector.tensor_scalar_add(z2[:], z_h[:, 0:1], float(K - KE))
            if KE > ecut:
                nc.vector.tensor_add(z2[:], z2[:], z_h[:, 1:2])
            rz = work.tile([nr, 1], F32, tag=f"rz{h}", name=f"rz{h}")
            nc.vector.reciprocal(rz[:], z2[:])
            nc.vector.tensor_scalar_mul(out_sb[:nr, :], po_h[:], rz[:])
            nc.sync.dma_start(out_d[r0 : r0 + nr, :], out_sb[:nr, :])

        # per-part PSUM score tiles -> disjoint banks, so the early epilogue
        # can read its scores while PE still accumulates the rest (same-bank
        # PE-W + engine-R is a hardware race)
        sc_h0 = scp.tile([H0, KE], F32, tag="sc_ps", name="sc_h0")
        N_BIAS = 4  # head rows on the no-DVE bias path while the pipe fills
        for q in range(N_BIAS):
            bias_tanh_single(q, sc_h0, chunked=(q < 2))
        q = N_BIAS
        for ng in group_sizes(H0 - N_BIAS):
            q_group_block(q, ng, lambda q_: sc_h0)
            q += ng
        po_h0 = pop.tile([H0, D], F32, tag="po", name="po_h0")
        epilogue_part(0, sc_h0, po_h0, 0, H0)
        sc_h1 = pp.tile([H1, KE], F32, tag="kp_ps", name="sc_h1")
        for ng in group_sizes(H1, deramp=True):
            q_group_block(q, ng, lambda q_: sc_h1)
            q += ng
        # keep the PE HAM-warm through the h1 mask/exp window so the attn@V
        # matmuls run at 2.4 GHz (PE idle > one MID window re-throttles it)
        wu2 = pop.tile([128, 512], F32, tag="po", name="wu2")
        for i in range(6):
            nc.tensor.matmul(wu2[:], wu_in[:, :128], wu_in[:], start=True, stop=True)
        po_h1 = pp.tile([H1, D], F32, tag="kp_ps", name="po_h1")
        epilogue_part(1, sc_h1, po_h1, H0, H1)

    nc.compile()
    return nc


_GRAPH_CACHE: dict[int, bass.Bass] = {}
_LAST_RESULTS = None


def _get_graph(KE: int) -> bass.Bass:
    if KE not in _GRAPH_CACHE:
        _GRAPH_CACHE[KE] = build_graph(KE)
    return _GRAPH_CACHE[KE]


def _sbuf_pack(mat_T):
    """[R*128, N] -> [128, R*N]: SBUF image with d-chunks along columns."""
    R = mat_T.shape[0] // 128
    return np.ascontiguousarray(
        mat_T.reshape(R, 128, -1).transpose(1, 0, 2).reshape(128, -1)
    )


def make_in_maps(queries, keys, values, Wq, Wk, wv, valid_lens, KE):
    wvwin = np.zeros((H, 128), BF16_NP)
    wvwin[:, 30] = wv.astype(BF16_NP)
    wvwin[:, 64 + 31] = wv.astype(BF16_NP)
    col = np.arange(KE)
    # k-chunk-major packing for kT: [128, sum_ci DC*w_ci]
    k_chunks = [(s, min(512, KE - s)) for s in range(0, KE, 512)]
    in_maps = []
    for c in range(N_CORES):
        b, qh = divmod(c, 2)
        mask_row = (col < int(valid_lens[b])).astype(np.float32)
        kT = keys[b, :KE, :].T.astype(BF16_NP)  # [D, KE]
        kT_packed = np.concatenate(
            [_sbuf_pack(kT[:, s : s + w]) for s, w in k_chunks], axis=1
        )
        sy_small = np.concatenate(
            [_sbuf_pack(Wk.astype(BF16_NP)), wvwin], axis=1
        )
        sc_small = np.concatenate(
            [
                _sbuf_pack(queries[b, qh * QS : (qh + 1) * QS, :].T.astype(BF16_NP)),
                _sbuf_pack(Wq.astype(BF16_NP)),
            ],
            axis=1,
        )
        in_maps.append(
            {
                "kT": np.ascontiguousarray(kT_packed),
                "v": _sbuf_pack(values[b].astype(BF16_NP)),
                "sy_small": np.ascontiguousarray(sy_small),
                "sc_small": np.ascontiguousarray(sc_small),
                "mask": np.ascontiguousarray(
                    np.broadcast_to(mask_row, (96, KE)).astype(BF16_NP)
                ),
            }
        )
    return in_maps


def kernel(queries, keys, values, Wq, Wk, wv, valid_lens, **run_kwargs):
    queries = np.asarray(queries, np.float32)
    keys = np.asarray(keys, np.float32)
    values = np.asarray(values, np.float32)
    Wq = np.asarray(Wq, np.float32)
    Wk = np.asarray(Wk, np.float32)
    wv = np.asarray(wv, np.float32)
    valid_lens = np.asarray(valid_lens, np.int32)

    KE = int(-(-int(valid_lens.max()) // 128) * 128)
    KE = max(128, min(K, KE))

    nc = _get_graph(KE)
    in_maps = make_in_maps(queries, keys, values, Wq, Wk, wv, valid_lens, KE)
    res = run_bass_kernel_spmd(
        nc, in_maps, core_ids=list(range(N_CORES)), **run_kwargs
    )
    global _LAST_RESULTS
    _LAST_RESULTS = res
    out = np.empty((B, Q, D), np.float32)
    for c in range(N_CORES):
        b, qh = divmod(c, 2)
        out[b, qh * QS : (qh + 1) * QS, :] = res.results[c]["out"]
    return out


# revision 76
# speedup vs baseline: 1.0091x; 1.0091x over previous
"""AdditiveAttention Trainium2 kernel (8 NeuronCores, SPMD, no collectives).

reference:
    q = queries @ Wq               (B,Q,H)
    k = keys @ Wk                  (B,K,H)
    scores[b,q,k] = sum_h wv[h] * tanh(q[b,q,h] + k[b,k,h])
    masked = where(arange(K) < valid_lens[b], scores, 0.0)
    attn = softmax(masked, -1)      # masked cols contribute e^0 = 1
    out = attn @ values             (B,Q,D)

Sharding: core c = (b, q_half) -> computes out[b, qh*128:(qh+1)*128, :].
Each core owns 128 queries x full K of one batch. Purely data-parallel,
no collectives, no cross-core reduction (strictly better here than the
flash-style K-split: each output element is computed exactly once).

Per-core structure (h=H=128 on partitions for the score stage;
ScalarE's tanh throughput, 1 elem/lane/cycle @ 1.2 GHz, is the roofline):
  - kpT[h, k], qpT[h, q] via PE matmuls (bf16 in, f32 accum); all inputs
    arrive host-packed as exact SBUF images so DMA descriptors are maximal;
    k-chunk-0 of kT split across both HWDGE rings, v/mask DMAs deferred
    behind the last kT piece (they'd steal SDMA packets at startup)
  - head 4 query rows: bias-fused tanh straight from the kp PSUM (ScalarE
    per-partition bias, no VectorE dependency, first two split per k-chunk)
    so the stream starts right after the first kp projection chunk
  - remaining rows in groups of 8: VectorE broadcast-adds kpT + qpT[:, q]
    (tensor_scalar, per-partition scalar, f32 2x mode, ~87us - hidden),
    then ONE ScalarE tanh over the fused group ([128, 8*KE]) amortizing
    the ~228-cycle ACT instruction overhead; groups de-ramp ..4,2,2 at the
    very end so the last rows' matmuls trail a short tanh
  - per q: 2 PE matmuls with a 32-wide "sliding window" stationary operand
    (wv at column q%32, zeros elsewhere) accumulate that q's scores into
    row q%32 (psum col-group base 32*g) of the part's PSUM tile
    -> dense scores[q, k]; wv is never reloaded per row
  - two row parts (96/32) in SEPARATE PSUM banks (same-bank PE-write +
    engine-read is a hardware race): part-0's entire epilogue - mask
    multiply (masked logits -> 0), exp with accum_out giving the softmax
    denominator for free, PE transposes E -> E_T in two banks, attn@V
    matmuls, 1/Z normalize, output DMA - runs hidden under part-1's tanh
    stream; only the last 32 rows' epilogue trails the loop (with dummy
    PE matmuls keeping the HAM clock warm through its mask/exp window)
  - k >= KE tail of attn@V uses an all-ones stationary operand (exp(0)=1)

KE = ceil(max(valid_lens)/128)*128 <= K: columns >= KE are masked in every
batch, so tanh/exp work shrinks to KE columns (kernel specializes the
compiled graph to the runtime valid_lens, cached per KE).

Measured (8 cores, neuron-profile exec_time, chip at full 1.2 GHz clock):
~130-132us at KE=896, ~145-146us at KE=1024 (ScalarE-busy floor ~114us at
KE=1024, ~100us at KE=896; the ACT stream has zero mid-loop gaps, so the
rest is NEFF boot ~7us + critical-DMA landing ~5us + projection chain
~2.5us + 32-row trailing epilogue ~7.5us incl. ~4.5us Tile end-drain).
The chip intermittently downclocks whole runs to 1.0/0.9 GHz (+20-30%).
First working version was 170.6us. rel err ~3e-3 (bf16 tanh/matmul
operands, f32 accumulation everywhere).
"""

import sys

sys.path.insert(0, "/opt/trn_rl_repo")

from contextlib import ExitStack

import numpy as np
import ml_dtypes

import concourse.bass as bass
import concourse.mybir as mybir
import concourse.tile as tile
from concourse import bacc
from concourse.bass_utils import run_bass_kernel_spmd
from concourse.masks import make_identity
from concourse.tile_rust import add_dep_helper

B, Q, K, D, H = 4, 256, 1024, 512, 128
QS = Q // 2  # queries per core
N_CORES = 8
F32 = mybir.dt.float32
BF16 = mybir.dt.bfloat16
BF16_NP = np.dtype(ml_dtypes.bfloat16)
WU_MM = 4  # PE warmup matmuls under the DMA shadow (more would delay the
# kp projections queued behind them in PE's FIFO; kp/qp/head matmuls extend
# the busy window past the ~3.4us HAM threshold so the clock still warms)


def build_graph(KE: int) -> bass.Bass:
    assert KE % 128 == 0 and 128 <= KE <= K
    DC = D // 128  # contraction chunks for the projections
    # n-chunks (<=512) of the score/exp free axis
    k_chunks = [(s, min(512, KE - s)) for s in range(0, KE, 512)]
    KC128 = KE // 128
    VC = K // 128
    HQ = QS // 2  # epilogue half

    H0 = 96  # rows finished early (hidden under the tanh stream)
    H1 = QS - H0

    nc = bacc.Bacc("TRN2", target_bir_lowering=False, debug=False)

    # all inputs arrive host-packed as the exact SBUF image ([128, N],
    # contiguous per partition) so every DMA runs at max descriptor size.
    # kT is additionally packed k-chunk-major so each k-chunk half is a
    # contiguous column range (split across the two HWDGE rings).
    kT_d = nc.declare_dram_parameter("kT", [128, DC * KE], BF16, isOutput=False)
    v_d = nc.declare_dram_parameter("v", [128, VC * D], BF16, isOutput=False)
    # small critical inputs concatenated per HWDGE ring (one DMA receipt each):
    # sy_small = wk || wvwin (sliding windows: col 30 / col 64+31 = wv)
    # sc_small = qT || wq
    sy_d = nc.declare_dram_parameter("sy_small", [128, DC * H + 128], BF16, isOutput=False)
    sc_d = nc.declare_dram_parameter("sc_small", [128, DC * QS + DC * H], BF16, isOutput=False)
    mask_d = nc.declare_dram_parameter("mask", [H0, KE], BF16, isOutput=False)
    out_d = nc.declare_dram_parameter("out", [QS, D], F32, isOutput=True)

    with tile.TileContext(nc) as tc, ExitStack() as ctx:
        const = ctx.enter_context(tc.tile_pool(name="const", bufs=1))
        work = ctx.enter_context(tc.tile_pool(name="work", bufs=1))
        tq_pool = ctx.enter_context(tc.tile_pool(name="tq", bufs=3))
        xa_pool = ctx.enter_context(tc.tile_pool(name="xa", bufs=3))
        pp = ctx.enter_context(tc.tile_pool(name="pp", bufs=1, space="PSUM"))
        scp = ctx.enter_context(tc.tile_pool(name="scp", bufs=1, space="PSUM"))
        tpp = ctx.enter_context(tc.tile_pool(name="tpp", bufs=2, space="PSUM"))
        pop = ctx.enter_context(tc.tile_pool(name="pop", bufs=1, space="PSUM"))

        # ---- load inputs (few big DMAs, spread over both HWDGE rings) ----
        kT_sb = const.tile([128, DC * KE], BF16, tag="kT")
        v_sb = const.tile([128, VC * D], BF16, tag="v")
        sy_sb = const.tile([128, DC * H + 128], BF16, tag="sy_small")
        sc_sb = const.tile([128, DC * QS + DC * H], BF16, tag="sc_small")
        mask_sb = const.tile([H0, KE], BF16, tag="mask")
        wk_sb = sy_sb[:, : DC * H]
        wvwin_sb = sy_sb[:, DC * H :]
        qT_sb = sc_sb[:, : DC * QS]
        wq_sb = sc_sb[:, DC * QS :]
        # smalls first (wk gates every kp matmul, qT/wq the bias path);
        # k-chunk-0 of kT is split across BOTH HWDGE rings so the first kp
        # chunk - which gates the first bias-fused tanh - lands in half the
        # time; chunk-major host packing keeps every piece contiguous
        kcut = DC * k_chunks[0][1]
        kq = kcut // 2
        nc.sync.dma_start(sy_sb[:], sy_d[:, :])
        nc.scalar.dma_start(sc_sb[:], sc_d[:, :])
        last_kt_sy = nc.sync.dma_start(kT_sb[:, :kq], kT_d[:, :kq])
        nc.scalar.dma_start(kT_sb[:, kq:kcut], kT_d[:, kq:kcut])
        if kcut < DC * KE:
            kq2 = (kcut + DC * KE) // 2
            last_kt_sy = nc.sync.dma_start(kT_sb[:, kcut:kq2], kT_d[:, kcut:kq2])
            nc.scalar.dma_start(kT_sb[:, kq2:], kT_d[:, kq2:])

        def kT_ci(ci, i):
            """d-chunk i of k-chunk ci, as packed: [base_ci + i*w, +w)."""
            base = DC * sum(w for _, w in k_chunks[:ci])
            w = k_chunks[ci][1]
            return kT_sb[:, base + i * w : base + (i + 1) * w]

        def v_c(i):
            return v_sb[:, i * D : (i + 1) * D]

        # ---- PE warmup burst (HAM un-throttle) under the DMA shadow ----
        wu_in = const.tile([128, 512], BF16, tag="wu_in")
        nc.gpsimd.memset(wu_in[:], 0.0)
        wu_ps = pop.tile([128, 512], F32, tag="po", name="wu_ps")
        for i in range(WU_MM):
            nc.tensor.matmul(
                wu_ps[:], wu_in[:, :128], wu_in[:], start=True, stop=True
            )

        # ---- projections ----
        # kp chunk 0 first, then qp (its qT/wq data lands before kT chunk 1
        # does, and the first bias-fused tanh needs qp_sb - keeping qp ahead
        # of kp chunk 1 in PE's in-order FIFO unblocks that tanh early),
        # then the remaining kp chunks
        kp_ps = pp.tile([H, KE], F32, tag="kp_ps")
        kp_sb = work.tile([H, KE], F32, tag="kp_sb")
        qp_ps = scp.tile([H, QS], F32, tag="sc_ps", name="qp_ps")
        qp_sb = work.tile([H, QS], F32, tag="qp_sb")
        for ci, (s, w) in enumerate(k_chunks):
            for i in range(DC):
                nc.tensor.matmul(
                    kp_ps[:, s : s + w],
                    wk_sb[:, i * H : (i + 1) * H],
                    kT_ci(ci, i),
                    start=(i == 0),
                    stop=(i == DC - 1),
                )
            nc.vector.tensor_copy(kp_sb[:, s : s + w], kp_ps[:, s : s + w])
            if ci == 0:
                for i in range(DC):
                    nc.tensor.matmul(
                        qp_ps[:],
                        wq_sb[:, i * H : (i + 1) * H],
                        qT_sb[:, i * QS : (i + 1) * QS],
                        start=(i == 0),
                        stop=(i == DC - 1),
                    )
                nc.vector.tensor_copy(qp_sb[:], qp_ps[:])

        # v/mask are not needed until the epilogue (~110us in): defer their
        # DMA triggers behind the last SYNC-ring kT piece so their 1.2MB
        # doesn't steal SDMA packets from the critical loads. They must sit
        # on the SYNC sequencer: a waiting trigger stalls its sequencer, and
        # scalar's sequencer has the whole tanh stream queued behind it
        # (sync has nothing until the output DMAs)
        vd = nc.sync.dma_start(v_sb[:], v_d[:, :])
        add_dep_helper(vd.ins, last_kt_sy.ins, reason="defer v dma")
        md = nc.sync.dma_start(mask_sb[:], mask_d[:, :])
        add_dep_helper(md.ins, last_kt_sy.ins, reason="defer mask dma")

        ident = const.tile([128, 128], BF16, tag="ident")
        make_identity(nc, ident[:])
        ones_sb = const.tile([128, 128], BF16, tag="ones")
        nc.gpsimd.memset(ones_sb[:], 1.0)

        et_sb = work.tile([128, KC128 * 128], BF16, tag="et_sb")
        out_sb = work.tile([QS, D], F32, tag="out_sb")

        def scores_mm(q, tq_ap, sc_h):
            """score scatter matmuls for one query row from its tanh slice."""
            g, r = divmod(q if q < H0 else q - H0, 32)
            off = (30 - r) if r % 2 == 0 else (64 + 31 - r)
            win = wvwin_sb[:, off : off + 32]
            for s, w in k_chunks:
                nc.tensor.matmul(
                    sc_h[g * 32 : (g + 1) * 32, s : s + w],
                    win,
                    tq_ap[:, s : s + w],
                    start=(r == 0),
                    stop=(r == 31),
                    tile_position=(0, g * 32),
                )

        QG = 8  # max queries fused per ScalarE tanh instruction

        def group_sizes(n, deramp=False):
            """small first group then 8s; de-ramp ..4,2,1,1 at the tail so the
            last rows' score matmuls trail a single-row tanh"""
            tail = [4, 2, 1, 1] if (deramp and n >= 2 * QG) else []
            rem = n - sum(tail)
            sizes = [rem % QG] if rem % QG else []
            sizes += [QG] * (rem // QG)
            return sizes + tail

        def bias_tanh_single(q, sc_h, chunked):
            """Head query rows: bias-fused tanh straight from the kp PSUM -
            no DVE-add dependency, so the stream starts right after the kp
            matmuls (chunked: right after the FIRST kp chunk, overlapping the
            second chunk's DMA + matmuls in the other PSUM bank)."""
            tq = tq_pool.tile([H, QG * KE], BF16, tag="tq", name="tq")
            for s, w in k_chunks if chunked else [(0, KE)]:
                nc.scalar.activation(
                    tq[:, s : s + w],
                    kp_ps[:, s : s + w],
                    mybir.ActivationFunctionType.Tanh,
                    bias=qp_sb[:, q : q + 1],
                )
            scores_mm(q, tq[:, :KE], sc_h)

        def q_group_block(q0, ng, sc_of):
            """Broadcast-add on DVE (per-partition scalar), pure tanh on
            ScalarE over a fused group of query rows (amortizes the ~228-cycle
            ACT per-instruction overhead), then the score matmuls."""
            xa = xa_pool.tile([H, QG * KE], F32, tag="xa", name="xa")
            for j in range(ng):
                nc.vector.tensor_scalar_add(
                    xa[:, j * KE : (j + 1) * KE],
                    kp_sb[:],
                    qp_sb[:, q0 + j : q0 + j + 1],
                )
            tq = tq_pool.tile([H, QG * KE], BF16, tag="tq", name="tq")
            nc.scalar.activation(
                tq[:, : ng * KE], xa[:, : ng * KE], mybir.ActivationFunctionType.Tanh
            )
            for j in range(ng):
                scores_mm(q0 + j, tq[:, j * KE : (j + 1) * KE], sc_of(q0 + j))

        def epilogue_part(h, sc_h, po_h, r0, nr):
            """mask + exp + transpose + attn@V + normalize + store for query
            rows [r0, r0+nr).

            All tiles here live on partitions 0:nr (engines cannot shift
            partitions); the q-offset reappears as a column offset in et_sb
            and as the DRAM row offset of the output DMA.
            """
            msk_h = work.tile([nr, KE], F32, tag=f"msk{h}", name=f"msk{h}")
            e_h = work.tile([nr, KE], BF16, tag=f"e{h}", name=f"e{h}")
            z_h = work.tile([nr, 2], F32, tag=f"z{h}", name=f"z{h}")
            # mask+exp in k-halves so exp(half0) overlaps mask(half1)
            ecut = k_chunks[0][1]
            for ei, (es, ew) in enumerate([(0, ecut), (ecut, KE - ecut)]):
                if ew <= 0:
                    continue
                nc.vector.tensor_mul(
                    msk_h[:, es : es + ew],
                    sc_h[:, es : es + ew],
                    mask_sb[:nr, es : es + ew],
                )
                nc.scalar.activation(
                    e_h[:, es : es + ew],
                    msk_h[:, es : es + ew],
                    mybir.ActivationFunctionType.Exp,
                    accum_out=z_h[:, ei : ei + 1],
                )
            # transposes in two waves over two PSUM banks; each wave's evac
            # (DVE) overlaps the other wave's transposes (PE)
            W0 = KC128 // 2
            waves = [(0, W0), (W0, KC128 - W0)] if W0 else [(0, KC128)]
            for c0, ncw in waves:
                tp = tpp.tile([128, max(KC128 - W0, 1) * nr], BF16, tag="tp", name="tp")
                for j in range(ncw):
                    nc.tensor.transpose(
                        tp[:, j * nr : (j + 1) * nr],
                        e_h[:, (c0 + j) * 128 : (c0 + j + 1) * 128],
                        ident[:nr, :nr],
                    )
                nc.vector.tensor_copy(
                    et_sb[:]
                    .rearrange("p (c n) -> p c n", c=KC128)[
                        :, c0 : c0 + ncw, r0 : r0 + nr
                    ],
                    tp[:, : ncw * nr].rearrange("p (c n) -> p c n", c=ncw),
                )
            # attn @ V (tail k-chunks beyond KE use ones: exp(0) = 1)
            for c in range(VC):
                lhsT = (
                    et_sb[:, c * 128 + r0 : c * 128 + r0 + nr]
                    if c < KC128
                    else ones_sb[:, :nr]
                )
                nc.tensor.matmul(
                    po_h[:],
                    lhsT,
                    v_c(c),
                    start=(c == 0),
                    stop=(c == VC - 1),
                )
            # normalize + store (partition-aligned: po_h/z_h live on 0:nr)
            z2 = work.tile([nr, 1], F32, tag=f"z2_{h}", name=f"z2_{h}")
            nc.vector.tensor_scalar_add(z2[:], z_h[:, 0:1], float(K - KE))
            if KE > ecut:
                nc.vector.tensor_add(z2[:], z2[:], z_h[:, 1:2])
            rz = work.tile([nr, 1], F32, tag=f"rz{h}", name=f"rz{h}")
            nc.vector.reciprocal(rz[:], z2[:])
            nc.vector.tensor_scalar_mul(out_sb[:nr, :], po_h[:], rz[:])
            nc.sync.dma_start(out_d[r0 : r0 + nr, :], out_sb[:nr, :])

        # per-part PSUM score tiles -> disjoint banks, so the early epilogue
        # can read its scores while PE still accumulates the rest (same-bank
        # PE-W + engine-R is a hardware race)
        sc_h0 = scp.tile([H0, KE], F32, tag="sc_ps", name="sc_h0")
        N_BIAS = 4  # head rows on the no-DVE bias path while the pipe fills
        # rows 0-1: chunk-MAJOR emission so both chunk-0 tanh pieces run
        # back-to-back in ACT's FIFO before either waits on kp chunk 1
        tq01 = [
            tq_pool.tile([H, QG * KE], BF16, tag="tq", name=f"tq_h{j}")
            for j in range(2)
        ]
        for s, w in k_chunks:
            for j in range(2):
                nc.scalar.activation(
                    tq01[j][:, s : s + w],
                    kp_ps[:, s : s + w],
                    mybir.ActivationFunctionType.Tanh,
                    bias=qp_sb[:, j : j + 1],
                )
        for j in range(2):
            scores_mm(j, tq01[j][:, :KE], sc_h0)
        for q in range(2, N_BIAS):
            bias_tanh_single(q, sc_h0, chunked=False)
        q = N_BIAS
        for ng in group_sizes(H0 - N_BIAS):
            q_group_block(q, ng, lambda q_: sc_h0)
            q += ng
        po_h0 = pop.tile([H0, D], F32, tag="po", name="po_h0")
        epilogue_part(0, sc_h0, po_h0, 0, H0)
        sc_h1 = pp.tile([H1, KE], F32, tag="kp_ps", name="sc_h1")
        for ng in group_sizes(H1, deramp=True):
            q_group_block(q, ng, lambda q_: sc_h1)
            q += ng
        # keep the PE HAM-warm through the h1 mask/exp window so the attn@V
        # matmuls run at 2.4 GHz (PE idle > one MID window re-throttles it)
        wu2 = pop.tile([128, 512], F32, tag="po", name="wu2")
        for i in range(6):
            nc.tensor.matmul(wu2[:], wu_in[:, :128], wu_in[:], start=True, stop=True)
        po_h1 = pp.tile([H1, D], F32, tag="kp_ps", name="po_h1")
        epilogue_part(1, sc_h1, po_h1, H0, H1)

    nc.compile()
    return nc


_GRAPH_CACHE: dict[int, bass.Bass] = {}
_LAST_RESULTS = None


def _get_graph(KE: int) -> bass.Bass:
    if KE not in _GRAPH_CACHE:
        _GRAPH_CACHE[KE] = build_graph(KE)
    return _GRAPH_CACHE[KE]


def _sbuf_pack(mat_T):
    """[R*128, N] -> [128, R*N]: SBUF image with d-chunks along columns."""
    R = mat_T.shape[0] // 128
    return np.ascontiguousarray(
        mat_T.reshape(R, 128, -1).transpose(1, 0, 2).reshape(128, -1)
    )


def make_in_maps(queries, keys, values, Wq, Wk, wv, valid_lens, KE):
    wvwin = np.zeros((H, 128), BF16_NP)
    wvwin[:, 30] = wv.astype(BF16_NP)
    wvwin[:, 64 + 31] = wv.astype(BF16_NP)
    col = np.arange(KE)
    # k-chunk-major packing for kT: [128, sum_ci DC*w_ci]
    k_chunks = [(s, min(512, KE - s)) for s in range(0, KE, 512)]
    in_maps = []
    for c in range(N_CORES):
        b, qh = divmod(c, 2)
        mask_row = (col < int(valid_lens[b])).astype(np.float32)
        kT = keys[b, :KE, :].T.astype(BF16_NP)  # [D, KE]
        kT_packed = np.concatenate(
            [_sbuf_pack(kT[:, s : s + w]) for s, w in k_chunks], axis=1
        )
        sy_small = np.concatenate(
            [_sbuf_pack(Wk.astype(BF16_NP)), wvwin], axis=1
        )
        sc_small = np.concatenate(
            [
                _sbuf_pack(queries[b, qh * QS : (qh + 1) * QS, :].T.astype(BF16_NP)),
                _sbuf_pack(Wq.astype(BF16_NP)),
            ],
            axis=1,
        )
        in_maps.append(
            {
                "kT": np.ascontiguousarray(kT_packed),
                "v": _sbuf_pack(values[b].astype(BF16_NP)),
                "sy_small": np.ascontiguousarray(sy_small),
                "sc_small": np.ascontiguousarray(sc_small),
                "mask": np.ascontiguousarray(
                    np.broadcast_to(mask_row, (96, KE)).astype(BF16_NP)
                ),
            }
        )
    return in_maps


def kernel(queries, keys, values, Wq, Wk, wv, valid_lens, **run_kwargs):
    queries = np.asarray(queries, np.float32)
    keys = np.asarray(keys, np.float32)
    values = np.asarray(values, np.float32)
    Wq = np.asarray(Wq, np.float32)
    Wk = np.asarray(Wk, np.float32)
    wv = np.asarray(wv, np.float32)
    valid_lens = np.asarray(valid_lens, np.int32)

    KE = int(-(-int(valid_lens.max()) // 128) * 128)
    KE = max(128, min(K, KE))

    nc = _get_graph(KE)
    in_maps = make_in_maps(queries, keys, values, Wq, Wk, wv, valid_lens, KE)
    res = run_bass_kernel_spmd(
        nc, in_maps, core_ids=list(range(N_CORES)), **run_kwargs
    )
    global _LAST_RESULTS
    _LAST_RESULTS = res
    out = np.empty((B, Q, D), np.float32)
    for c in range(N_CORES):
        b, qh = divmod(c, 2)
        out[b, qh * QS : (qh + 1) * QS, :] = res.results[c]["out"]
    return out


# revision 80
# speedup vs baseline: 1.0242x; 1.0150x over previous
"""AdditiveAttention Trainium2 kernel (8 NeuronCores, SPMD, no collectives).

reference:
    q = queries @ Wq               (B,Q,H)
    k = keys @ Wk                  (B,K,H)
    scores[b,q,k] = sum_h wv[h] * tanh(q[b,q,h] + k[b,k,h])
    masked = where(arange(K) < valid_lens[b], scores, 0.0)
    attn = softmax(masked, -1)      # masked cols contribute e^0 = 1
    out = attn @ values             (B,Q,D)

Sharding: core c = (b, q_half) -> computes out[b, qh*128:(qh+1)*128, :].
Each core owns 128 queries x full K of one batch. Purely data-parallel,
no collectives, no cross-core reduction (strictly better here than the
flash-style K-split: each output element is computed exactly once).

Per-core structure (h=H=128 on partitions for the score stage;
ScalarE's tanh throughput, 1 elem/lane/cycle @ 1.2 GHz, is the roofline):
  - kpT[h, k], qpT[h, q] via PE matmuls (bf16 in, f32 accum); all inputs
    arrive host-packed as exact SBUF images so DMA descriptors are maximal;
    k-chunk-0 of kT split across both HWDGE rings, v/mask DMAs deferred
    behind the last kT piece (they'd steal SDMA packets at startup)
  - head 4 query rows: bias-fused tanh straight from the kp PSUM (ScalarE
    per-partition bias, no VectorE dependency, first two split per k-chunk)
    so the stream starts right after the first kp projection chunk
  - remaining rows in groups of 8: VectorE broadcast-adds kpT + qpT[:, q]
    (tensor_scalar, per-partition scalar, f32 2x mode, ~87us - hidden),
    then ONE ScalarE tanh over the fused group ([128, 8*KE]) amortizing
    the ~228-cycle ACT instruction overhead; groups de-ramp ..4,2,2 at the
    very end so the last rows' matmuls trail a short tanh
  - per q: 2 PE matmuls with a 32-wide "sliding window" stationary operand
    (wv at column q%32, zeros elsewhere) accumulate that q's scores into
    row q%32 (psum col-group base 32*g) of the part's PSUM tile
    -> dense scores[q, k]; wv is never reloaded per row
  - two row parts (96/32) in SEPARATE PSUM banks (same-bank PE-write +
    engine-read is a hardware race): part-0's entire epilogue - mask
    multiply (masked logits -> 0), exp with accum_out giving the softmax
    denominator for free, PE transposes E -> E_T in two banks, attn@V
    matmuls, 1/Z normalize, output DMA - runs hidden under part-1's tanh
    stream; only the last 32 rows' epilogue trails the loop (with dummy
    PE matmuls keeping the HAM clock warm through its mask/exp window)
  - k >= KE tail of attn@V uses an all-ones stationary operand (exp(0)=1)

KE = ceil(max(valid_lens)/128)*128 <= K: columns >= KE are masked in every
batch, so tanh/exp work shrinks to KE columns (kernel specializes the
compiled graph to the runtime valid_lens, cached per KE).

Measured (8 cores, neuron-profile exec_time, chip at full 1.2 GHz clock):
~130-132us at KE=896, ~145-146us at KE=1024 (ScalarE-busy floor ~114us at
KE=1024, ~100us at KE=896; the ACT stream has zero mid-loop gaps, so the
rest is NEFF boot ~7us + critical-DMA landing ~5us + projection chain
~2.5us + 32-row trailing epilogue ~7.5us incl. ~4.5us Tile end-drain).
The chip intermittently downclocks whole runs to 1.0/0.9 GHz (+20-30%).
First working version was 170.6us. rel err ~3e-3 (bf16 tanh/matmul
operands, f32 accumulation everywhere).
"""

import sys

sys.path.insert(0, "/opt/trn_rl_repo")

from contextlib import ExitStack

import numpy as np
import ml_dtypes

import concourse.bass as bass
import concourse.mybir as mybir
import concourse.tile as tile
from concourse import bacc
from concourse.bass_utils import run_bass_kernel_spmd
from concourse.masks import make_identity
from concourse.tile_rust import add_dep_helper

B, Q, K, D, H = 4, 256, 1024, 512, 128
QS = Q // 2  # queries per core
N_CORES = 8
F32 = mybir.dt.float32
BF16 = mybir.dt.bfloat16
BF16_NP = np.dtype(ml_dtypes.bfloat16)
WU_MM = 4  # PE warmup matmuls under the DMA shadow (more would delay the
# kp projections queued behind them in PE's FIFO; kp/qp/head matmuls extend
# the busy window past the ~3.4us HAM threshold so the clock still warms)


def build_graph(KE: int) -> bass.Bass:
    assert KE % 128 == 0 and 128 <= KE <= K
    DC = D // 128  # contraction chunks for the projections
    # n-chunks (<=512) of the score/exp free axis
    k_chunks = [(s, min(512, KE - s)) for s in range(0, KE, 512)]
    KC128 = KE // 128
    VC = K // 128
    HQ = QS // 2  # epilogue half

    H0 = 96  # rows finished early (hidden under the tanh stream)
    H1 = QS - H0

    nc = bacc.Bacc("TRN2", target_bir_lowering=False, debug=False)

    # all inputs arrive host-packed as the exact SBUF image ([128, N],
    # contiguous per partition) so every DMA runs at max descriptor size.
    # kT is additionally packed k-chunk-major so each k-chunk half is a
    # contiguous column range (split across the two HWDGE rings).
    kT_d = nc.declare_dram_parameter("kT", [128, DC * KE], BF16, isOutput=False)
    v_d = nc.declare_dram_parameter("v", [128, VC * D], BF16, isOutput=False)
    # small critical inputs concatenated per HWDGE ring (one DMA receipt each):
    # sy_small = wk || wvwin (sliding windows: col 30 / col 64+31 = wv)
    # sc_small = qT || wq
    sy_d = nc.declare_dram_parameter("sy_small", [128, DC * H + 128], BF16, isOutput=False)
    sc_d = nc.declare_dram_parameter("sc_small", [128, DC * QS + DC * H], BF16, isOutput=False)
    mask_d = nc.declare_dram_parameter("mask", [H0, KE], BF16, isOutput=False)
    out_d = nc.declare_dram_parameter("out", [QS, D], F32, isOutput=True)

    with tile.TileContext(nc) as tc, ExitStack() as ctx:
        const = ctx.enter_context(tc.tile_pool(name="const", bufs=1))
        work = ctx.enter_context(tc.tile_pool(name="work", bufs=1))
        tq_pool = ctx.enter_context(tc.tile_pool(name="tq", bufs=3))
        xa_pool = ctx.enter_context(tc.tile_pool(name="xa", bufs=3))
        pp = ctx.enter_context(tc.tile_pool(name="pp", bufs=1, space="PSUM"))
        scp = ctx.enter_context(tc.tile_pool(name="scp", bufs=1, space="PSUM"))
        tpp = ctx.enter_context(tc.tile_pool(name="tpp", bufs=2, space="PSUM"))
        pop = ctx.enter_context(tc.tile_pool(name="pop", bufs=1, space="PSUM"))

        # ---- load inputs (few big DMAs, spread over both HWDGE rings) ----
        kT_sb = const.tile([128, DC * KE], BF16, tag="kT")
        v_sb = const.tile([128, VC * D], BF16, tag="v")
        sy_sb = const.tile([128, DC * H + 128], BF16, tag="sy_small")
        sc_sb = const.tile([128, DC * QS + DC * H], BF16, tag="sc_small")
        mask_sb = const.tile([H0, KE], BF16, tag="mask")
        wk_sb = sy_sb[:, : DC * H]
        wvwin_sb = sy_sb[:, DC * H :]
        qT_sb = sc_sb[:, : DC * QS]
        wq_sb = sc_sb[:, DC * QS :]
        # smalls first (wk gates every kp matmul, qT/wq the bias path);
        # k-chunk-0 of kT is split across BOTH HWDGE rings so the first kp
        # chunk - which gates the first bias-fused tanh - lands in half the
        # time; chunk-major host packing keeps every piece contiguous
        kcut = DC * k_chunks[0][1]
        kq = kcut // 2
        nc.sync.dma_start(sy_sb[:], sy_d[:, :])
        nc.scalar.dma_start(sc_sb[:], sc_d[:, :])
        last_kt_sy = nc.sync.dma_start(kT_sb[:, :kq], kT_d[:, :kq])
        nc.scalar.dma_start(kT_sb[:, kq:kcut], kT_d[:, kq:kcut])
        if kcut < DC * KE:
            kq2 = (kcut + DC * KE) // 2
            last_kt_sy = nc.sync.dma_start(kT_sb[:, kcut:kq2], kT_d[:, kcut:kq2])
            nc.scalar.dma_start(kT_sb[:, kq2:], kT_d[:, kq2:])

        def kT_ci(ci, i):
            """d-chunk i of k-chunk ci, as packed: [base_ci + i*w, +w)."""
            base = DC * sum(w for _, w in k_chunks[:ci])
            w = k_chunks[ci][1]
            return kT_sb[:, base + i * w : base + (i + 1) * w]

        def v_c(i):
            return v_sb[:, i * D : (i + 1) * D]

        # ---- PE warmup burst (HAM un-throttle) under the DMA shadow ----
        wu_in = const.tile([128, 512], BF16, tag="wu_in")
        nc.gpsimd.memset(wu_in[:], 0.0)
        wu_ps = pop.tile([128, 512], F32, tag="po", name="wu_ps")
        for i in range(WU_MM):
            nc.tensor.matmul(
                wu_ps[:], wu_in[:, :128], wu_in[:], start=True, stop=True
            )

        # ---- projections ----
        # kp chunk 0 first, then qp (its qT/wq data lands before kT chunk 1
        # does, and the first bias-fused tanh needs qp_sb - keeping qp ahead
        # of kp chunk 1 in PE's in-order FIFO unblocks that tanh early),
        # then the remaining kp chunks
        # one PSUM tensor PER k-chunk: Tile couples a reader to all of a
        # tensor's writers, so the head bias-tanh pieces (chunk-0 only)
        # must not share a tensor with the later chunk-1 matmuls
        kp_ps = [
            pp.tile([H, w], F32, tag=("kp_c0" if ci == 0 else f"kp_c{ci}"),
                    name=f"kp_ps{ci}")
            for ci, (s, w) in enumerate(k_chunks)
        ]
        kp_sb = work.tile([H, KE], F32, tag="kp_sb")
        qp_ps = scp.tile([H, QS], F32, tag="sc_ps", name="qp_ps")
        qp_sb = work.tile([H, QS], F32, tag="qp_sb")
        for ci, (s, w) in enumerate(k_chunks):
            for i in range(DC):
                nc.tensor.matmul(
                    kp_ps[ci][:, :w],
                    wk_sb[:, i * H : (i + 1) * H],
                    kT_ci(ci, i),
                    start=(i == 0),
                    stop=(i == DC - 1),
                )
            nc.vector.tensor_copy(kp_sb[:, s : s + w], kp_ps[ci][:, :w])
            if ci == 0:
                for i in range(DC):
                    nc.tensor.matmul(
                        qp_ps[:],
                        wq_sb[:, i * H : (i + 1) * H],
                        qT_sb[:, i * QS : (i + 1) * QS],
                        start=(i == 0),
                        stop=(i == DC - 1),
                    )
                nc.vector.tensor_copy(qp_sb[:], qp_ps[:])

        # v/mask are not needed until the epilogue (~110us in): defer their
        # DMA triggers behind the last SYNC-ring kT piece so their 1.2MB
        # doesn't steal SDMA packets from the critical loads. They must sit
        # on the SYNC sequencer: a waiting trigger stalls its sequencer, and
        # scalar's sequencer has the whole tanh stream queued behind it
        # (sync has nothing until the output DMAs)
        vd = nc.sync.dma_start(v_sb[:], v_d[:, :])
        add_dep_helper(vd.ins, last_kt_sy.ins, reason="defer v dma")
        md = nc.sync.dma_start(mask_sb[:], mask_d[:, :])
        add_dep_helper(md.ins, last_kt_sy.ins, reason="defer mask dma")

        ident = const.tile([128, 128], BF16, tag="ident")
        make_identity(nc, ident[:])
        ones_sb = const.tile([128, 128], BF16, tag="ones")
        nc.gpsimd.memset(ones_sb[:], 1.0)

        et_sb = work.tile([128, KC128 * 128], BF16, tag="et_sb")
        out_sb = work.tile([QS, D], F32, tag="out_sb")

        def scores_mm(q, tq_ap, sc_h):
            """score scatter matmuls for one query row from its tanh slice."""
            g, r = divmod(q if q < H0 else q - H0, 32)
            off = (30 - r) if r % 2 == 0 else (64 + 31 - r)
            win = wvwin_sb[:, off : off + 32]
            for s, w in k_chunks:
                nc.tensor.matmul(
                    sc_h[g * 32 : (g + 1) * 32, s : s + w],
                    win,
                    tq_ap[:, s : s + w],
                    start=(r == 0),
                    stop=(r == 31),
                    tile_position=(0, g * 32),
                )

        QG = 8  # max queries fused per ScalarE tanh instruction

        def group_sizes(n, deramp=False):
            """small first group then 8s; de-ramp ..4,2,1,1 at the tail so the
            last rows' score matmuls trail a single-row tanh"""
            tail = [4, 2, 1, 1] if (deramp and n >= 2 * QG) else []
            rem = n - sum(tail)
            sizes = [rem % QG] if rem % QG else []
            sizes += [QG] * (rem // QG)
            return sizes + tail

        def bias_tanh_piece(q, tq, ci):
            """One k-chunk piece of a head row's bias-fused tanh straight
            from that chunk's kp PSUM tensor - no DVE-add dependency and no
            dependency on the other chunk's matmuls."""
            s, w = k_chunks[ci]
            nc.scalar.activation(
                tq[:, s : s + w],
                kp_ps[ci][:, :w],
                mybir.ActivationFunctionType.Tanh,
                bias=qp_sb[:, q : q + 1],
            )

        def q_group_block(q0, ng, sc_of):
            """Broadcast-add on DVE (per-partition scalar), pure tanh on
            ScalarE over a fused group of query rows (amortizes the ~228-cycle
            ACT per-instruction overhead), then the score matmuls."""
            xa = xa_pool.tile([H, QG * KE], F32, tag="xa", name="xa")
            for j in range(ng):
                nc.vector.tensor_scalar_add(
                    xa[:, j * KE : (j + 1) * KE],
                    kp_sb[:],
                    qp_sb[:, q0 + j : q0 + j + 1],
                )
            tq = tq_pool.tile([H, QG * KE], BF16, tag="tq", name="tq")
            nc.scalar.activation(
                tq[:, : ng * KE], xa[:, : ng * KE], mybir.ActivationFunctionType.Tanh
            )
            for j in range(ng):
                scores_mm(q0 + j, tq[:, j * KE : (j + 1) * KE], sc_of(q0 + j))

        def epilogue_part(h, sc_h, po_h, r0, nr):
            """mask + exp + transpose + attn@V + normalize + store for query
            rows [r0, r0+nr).

            All tiles here live on partitions 0:nr (engines cannot shift
            partitions); the q-offset reappears as a column offset in et_sb
            and as the DRAM row offset of the output DMA.
            """
            msk_h = work.tile([nr, KE], F32, tag=f"msk{h}", name=f"msk{h}")
            e_h = work.tile([nr, KE], BF16, tag=f"e{h}", name=f"e{h}")
            z_h = work.tile([nr, 2], F32, tag=f"z{h}", name=f"z{h}")
            # mask+exp in k-halves so exp(half0) overlaps mask(half1)
            ecut = k_chunks[0][1]
            for ei, (es, ew) in enumerate([(0, ecut), (ecut, KE - ecut)]):
                if ew <= 0:
                    continue
                nc.vector.tensor_mul(
                    msk_h[:, es : es + ew],
                    sc_h[:, es : es + ew],
                    mask_sb[:nr, es : es + ew],
                )
                nc.scalar.activation(
                    e_h[:, es : es + ew],
                    msk_h[:, es : es + ew],
                    mybir.ActivationFunctionType.Exp,
                    accum_out=z_h[:, ei : ei + 1],
                )
            # transposes in two waves over two PSUM banks; each wave's evac
            # (DVE) overlaps the other wave's transposes (PE)
            W0 = KC128 // 2
            waves = [(0, W0), (W0, KC128 - W0)] if W0 else [(0, KC128)]
            for c0, ncw in waves:
                tp = tpp.tile([128, max(KC128 - W0, 1) * nr], BF16, tag="tp", name="tp")
                for j in range(ncw):
                    nc.tensor.transpose(
                        tp[:, j * nr : (j + 1) * nr],
                        e_h[:, (c0 + j) * 128 : (c0 + j + 1) * 128],
                        ident[:nr, :nr],
                    )
                nc.vector.tensor_copy(
                    et_sb[:]
                    .rearrange("p (c n) -> p c n", c=KC128)[
                        :, c0 : c0 + ncw, r0 : r0 + nr
                    ],
                    tp[:, : ncw * nr].rearrange("p (c n) -> p c n", c=ncw),
                )
            # attn @ V (tail k-chunks beyond KE use ones: exp(0) = 1)
            for c in range(VC):
                lhsT = (
                    et_sb[:, c * 128 + r0 : c * 128 + r0 + nr]
                    if c < KC128
                    else ones_sb[:, :nr]
                )
                nc.tensor.matmul(
                    po_h[:],
                    lhsT,
                    v_c(c),
                    start=(c == 0),
                    stop=(c == VC - 1),
                )
            # normalize + store (partition-aligned: po_h/z_h live on 0:nr)
            z2 = work.tile([nr, 1], F32, tag=f"z2_{h}", name=f"z2_{h}")
            nc.vector.tensor_scalar_add(z2[:], z_h[:, 0:1], float(K - KE))
            if KE > ecut:
                nc.vector.tensor_add(z2[:], z2[:], z_h[:, 1:2])
            rz = work.tile([nr, 1], F32, tag=f"rz{h}", name=f"rz{h}")
            nc.vector.reciprocal(rz[:], z2[:])
            nc.vector.tensor_scalar_mul(out_sb[:nr, :], po_h[:], rz[:])
            nc.sync.dma_start(out_d[r0 : r0 + nr, :], out_sb[:nr, :])

        # per-part PSUM score tiles -> disjoint banks, so the early epilogue
        # can read its scores while PE still accumulates the rest (same-bank
        # PE-W + engine-R is a hardware race)
        sc_h0 = scp.tile([H0, KE], F32, tag="sc_ps", name="sc_h0")
        N_BIAS = 4  # head rows on the no-DVE bias path while the pipe fills
        # chunk-MAJOR emission: all chunk-0 tanh pieces run back-to-back in
        # ACT's FIFO before any piece waits on kp chunk 1
        tq_head = [
            tq_pool.tile([H, QG * KE], BF16, tag="tq", name=f"tq_h{j}")
            for j in range(2)
        ]
        for ci in range(len(k_chunks)):
            for j in range(N_BIAS):
                bias_tanh_piece(j, tq_head[j // 2][:, (j % 2) * KE :], ci)
        for j in range(N_BIAS):
            tqj = tq_head[j // 2][:, (j % 2) * KE :]
            scores_mm(j, tqj[:, :KE], sc_h0)
        q = N_BIAS
        for ng in group_sizes(H0 - N_BIAS):
            q_group_block(q, ng, lambda q_: sc_h0)
            q += ng
        po_h0 = pop.tile([H0, D], F32, tag="po", name="po_h0")
        epilogue_part(0, sc_h0, po_h0, 0, H0)
        sc_h1 = pp.tile([H1, KE], F32, tag="kp_c0", name="sc_h1")
        for ng in group_sizes(H1, deramp=True):
            q_group_block(q, ng, lambda q_: sc_h1)
            q += ng
        # keep the PE HAM-warm through the h1 mask/exp window so the attn@V
        # matmuls run at 2.4 GHz (PE idle > one MID window re-throttles it)
        wu2 = pop.tile([128, 512], F32, tag="po", name="wu2")
        for i in range(6):
            nc.tensor.matmul(wu2[:], wu_in[:, :128], wu_in[:], start=True, stop=True)
        po_h1 = pp.tile([H1, D], F32, tag="kp_c0", name="po_h1")
        epilogue_part(1, sc_h1, po_h1, H0, H1)

    nc.compile()
    return nc


_GRAPH_CACHE: dict[int, bass.Bass] = {}
_LAST_RESULTS = None


def _get_graph(KE: int) -> bass.Bass:
    if KE not in _GRAPH_CACHE:
        _GRAPH_CACHE[KE] = build_graph(KE)
    return _GRAPH_CACHE[KE]


def _sbuf_pack(mat_T):
    """[R*128, N] -> [128, R*N]: SBUF image with d-chunks along columns."""
    R = mat_T.shape[0] // 128
    return np.ascontiguousarray(
        mat_T.reshape(R, 128, -1).transpose(1, 0, 2).reshape(128, -1)
    )


def make_in_maps(queries, keys, values, Wq, Wk, wv, valid_lens, KE):
    wvwin = np.zeros((H, 128), BF16_NP)
    wvwin[:, 30] = wv.astype(BF16_NP)
    wvwin[:, 64 + 31] = wv.astype(BF16_NP)
    col = np.arange(KE)
    # k-chunk-major packing for kT: [128, sum_ci DC*w_ci]
    k_chunks = [(s, min(512, KE - s)) for s in range(0, KE, 512)]
    in_maps = []
    for c in range(N_CORES):
        b, qh = divmod(c, 2)
        mask_row = (col < int(valid_lens[b])).astype(np.float32)
        kT = keys[b, :KE, :].T.astype(BF16_NP)  # [D, KE]
        kT_packed = np.concatenate(
            [_sbuf_pack(kT[:, s : s + w]) for s, w in k_chunks], axis=1
        )
        sy_small = np.concatenate(
            [_sbuf_pack(Wk.astype(BF16_NP)), wvwin], axis=1
        )
        sc_small = np.concatenate(
            [
                _sbuf_pack(queries[b, qh * QS : (qh + 1) * QS, :].T.astype(BF16_NP)),
                _sbuf_pack(Wq.astype(BF16_NP)),
            ],
            axis=1,
        )
        in_maps.append(
            {
                "kT": np.ascontiguousarray(kT_packed),
                "v": _sbuf_pack(values[b].astype(BF16_NP)),
                "sy_small": np.ascontiguousarray(sy_small),
                "sc_small": np.ascontiguousarray(sc_small),
                "mask": np.ascontiguousarray(
                    np.broadcast_to(mask_row, (96, KE)).astype(BF16_NP)
                ),
            }
        )
    return in_maps


def kernel(queries, keys, values, Wq, Wk, wv, valid_lens, **run_kwargs):
    queries = np.asarray(queries, np.float32)
    keys = np.asarray(keys, np.float32)
    values = np.asarray(values, np.float32)
    Wq = np.asarray(Wq, np.float32)
    Wk = np.asarray(Wk, np.float32)
    wv = np.asarray(wv, np.float32)
    valid_lens = np.asarray(valid_lens, np.int32)

    KE = int(-(-int(valid_lens.max()) // 128) * 128)
    KE = max(128, min(K, KE))

    nc = _get_graph(KE)
    in_maps = make_in_maps(queries, keys, values, Wq, Wk, wv, valid_lens, KE)
    res = run_bass_kernel_spmd(
        nc, in_maps, core_ids=list(range(N_CORES)), **run_kwargs
    )
    global _LAST_RESULTS
    _LAST_RESULTS = res
    out = np.empty((B, Q, D), np.float32)
    for c in range(N_CORES):
        b, qh = divmod(c, 2)
        out[b, qh * QS : (qh + 1) * QS, :] = res.results[c]["out"]
    return out
